# revision 1
# baseline (speedup 1.0000x reference)
"""Multi-head causal self-attention with RoPE, tensor-parallel over heads
across 8 Trainium2 NeuronCores.

Strategy (Megatron-style TP over heads):
  - Each core owns 2 of the 16 heads: rows [c*256,(c+1)*256) of Wq/Wk/Wv
    and the matching columns of Wo.
  - On-core: qT/kT projections in transposed [d, s] layout (natural matmul
    output layout), RoPE via a signed-permutation matmul + elementwise ops,
    v in natural [s, d] layout, causal attention with scores computed
    transposed (S^T = K Q^T, softmax sum via a ones-matmul, no running max
    needed -- scores are O(10) so exp() cannot overflow), then a partial
    output projection against the core's Wo column-slice.
  - Host sums the 8 partial outputs (this replaces the TP all-reduce).

All matmuls run on float32r operands (full-rate fp32 on the PE). The BIR
verifier requires float32r matmul inputs to be produced as float32r, so
DRAM-sourced operands are declared float32r and engine-produced operands
(RoPE'd q/k, exp(scores), v, u) are written with float32r output dtype.
"""

import sys

import numpy as np

B, S, DIM = 2, 2048, 2048
NUM_HEADS = 16
HD = 128
N_CORES = 8
HPC = NUM_HEADS // N_CORES  # heads per core
DLOC = HPC * HD             # per-core slice of the model dim
ROPE_BASE = 10000.0
QCH = 512                   # attention q-chunk / phase-3 out-chunk
SC1 = 256                   # phase-1 s-chunk

_PROGRAM_CACHE = {}


def _rope_tables_T(seq_len, head_dim):
    # match reference float32 arithmetic: inv_freq over even indices,
    # emb = cat(freqs, freqs); returned transposed [head_dim, seq_len]
    inv_freq = (
        1.0
        / (np.float32(ROPE_BASE)
           ** (np.arange(0, head_dim, 2, dtype=np.float32) / np.float32(head_dim)))
    ).astype(np.float32)
    t = np.arange(seq_len, dtype=np.float32)
    freqs = np.outer(t, inv_freq).astype(np.float32)      # [S, D/2]
    emb = np.concatenate([freqs, freqs], axis=-1)         # [S, D]
    return (
        np.ascontiguousarray(np.cos(emb).astype(np.float32).T),
        np.ascontiguousarray(np.sin(emb).astype(np.float32).T),
    )


def _rot_matrix_T(head_dim):
    # rotated = cat(-x[1::2], x[::2]) = R @ x; return R.T [D, D]
    d2 = head_dim // 2
    R = np.zeros((head_dim, head_dim), dtype=np.float32)
    for dp in range(d2):
        R[dp, 2 * dp + 1] = -1.0
    for dp in range(d2, head_dim):
        R[dp, 2 * (dp - d2)] = 1.0
    return np.ascontiguousarray(R.T)


def _causal_masks(qch):
    # masks[i][kk, qq] = 0 if 128*i + kk <= qq else -1e9 (additive, applied
    # to raw scores before exp, for the 4 diagonal k-chunks of each q-chunk)
    m = np.zeros((4, 128, qch), dtype=np.float32)
    kk = np.arange(128)[:, None]
    qq = np.arange(qch)[None, :]
    for i in range(4):
        m[i] = np.where(128 * i + kk <= qq, 0.0, -1e9).astype(np.float32)
    return m


def build_program(b=B, s=S, dim=DIM):
    """Builds the per-core SPMD Bass program (identical on every core)."""
    if "/opt/trn_rl_repo" not in sys.path:
        sys.path.insert(0, "/opt/trn_rl_repo")
    import concourse.bacc as bacc
    import concourse.mybir as mybir
    import concourse.tile as tile

    f32 = mybir.dt.float32
    f32r = mybir.dt.float32r
    EXP = mybir.ActivationFunctionType.Exp

    bs = b * s
    n_din = dim // 128          # contraction chunks for projections
    n_s1 = bs // SC1            # phase-1 s-chunks
    n_qc = s // QCH             # attention q-chunks per batch
    n_sc3 = bs // 128           # phase-3 row chunks
    n_oc = dim // QCH           # phase-3 out-column chunks
    scale = float(HD) ** -0.5

    nc = bacc.Bacc("TRN2", target_bir_lowering=False, debug=False)

    xT_d = nc.dram_tensor("xT", [dim, bs], f32r, kind="ExternalInput")
    wqT_d = nc.dram_tensor("wqT", [dim, DLOC], f32r, kind="ExternalInput")
    wkT_d = nc.dram_tensor("wkT", [dim, DLOC], f32r, kind="ExternalInput")
    wvT_d = nc.dram_tensor("wvT", [dim, DLOC], f32r, kind="ExternalInput")
    woT_d = nc.dram_tensor("woT", [DLOC, dim], f32r, kind="ExternalInput")
    cosT_d = nc.dram_tensor("cosT", [HD, bs], f32, kind="ExternalInput")
    sinT_d = nc.dram_tensor("sinT", [HD, bs], f32, kind="ExternalInput")
    rT_d = nc.dram_tensor("rT", [HD, HD], f32r, kind="ExternalInput")
    ones_d = nc.dram_tensor("ones", [HD, HD], f32r, kind="ExternalInput")
    masks_d = nc.dram_tensor("masks", [4, HD, QCH], mybir.dt.bfloat16, kind="ExternalInput")
    out_d = nc.dram_tensor("out", [dim, bs], f32, kind="ExternalOutput")

    with tile.TileContext(nc) as tc:
        with tc.tile_pool(name="persist", bufs=1) as persist:
            # transposed roped projections [d, head, b*s]; v natural [s, chunk, d]
            qT = persist.tile([128, HPC, bs], f32r)
            kT = persist.tile([128, HPC, bs], f32r)
            vS = persist.tile([128, bs // 128, DLOC], f32r)
            rTs = persist.tile([HD, HD], f32r)
            nc.sync.dma_start(out=rTs, in_=rT_d[:])
            ones = persist.tile([128, 128], f32r)
            nc.sync.dma_start(out=ones, in_=ones_d[:])
            masks_s = persist.tile([128, 4, QCH], mybir.dt.bfloat16)
            nc.sync.dma_start(out=masks_s, in_=masks_d.rearrange("i p q -> p i q"))
            woT_s = persist.tile([128, HPC, dim], f32r)
            nc.sync.dma_start(
                out=woT_s, in_=woT_d.rearrange("(h p) n -> p h n", p=128)
            )

            # ---------------- phase 1: qkv projections + RoPE ----------------
            with (
                tc.tile_pool(name="p1w", bufs=1) as p1w,
                tc.tile_pool(name="p1x", bufs=2) as p1x,
                tc.tile_pool(name="p1t", bufs=2) as p1t,
                tc.tile_pool(name="ps_qk", bufs=4, space="PSUM") as ps_qk,
                tc.tile_pool(name="ps_rot", bufs=2, space="PSUM") as ps_rot,
                tc.tile_pool(name="ps_v", bufs=2, space="PSUM") as ps_v,
            ):
                wq_s = p1w.tile([128, n_din, DLOC], f32r)
                wk_s = p1w.tile([128, n_din, DLOC], f32r)
                wv_s = p1w.tile([128, n_din, DLOC], f32r)
                # split weight loads so the first matmuls start as soon as the
                # first contraction chunks land (DMA queues run in parallel)
                gw = max(1, n_din // 4)
                for g0 in range(0, n_din, gw):
                    for w_t, w_d in ((wq_s, wqT_d), (wk_s, wkT_d), (wv_s, wvT_d)):
                        nc.sync.dma_start(
                            out=w_t[:, g0 : g0 + gw, :],
                            in_=w_d.rearrange("(c p) m -> p c m", p=128)[
                                :, g0 : g0 + gw, :
                            ],
                        )

                for si in range(n_s1):
                    s0 = si * SC1
                    xt = p1x.tile([128, n_din, SC1], f32r, tag="xt")
                    xsrc = xT_d[:, s0 : s0 + SC1].rearrange("(c p) s -> p c s", p=128)
                    nh = n_din // 2
                    nc.sync.dma_start(out=xt[:, :nh, :], in_=xsrc[:, :nh, :])
                    nc.sync.dma_start(out=xt[:, nh:, :], in_=xsrc[:, nh:, :])
                    cost = p1x.tile([128, SC1], f32, tag="cost")
                    nc.sync.dma_start(out=cost, in_=cosT_d[:, s0 : s0 + SC1])
                    sint = p1x.tile([128, SC1], f32, tag="sint")
                    nc.sync.dma_start(out=sint, in_=sinT_d[:, s0 : s0 + SC1])

                    for w_s, store in ((wq_s, qT), (wk_s, kT)):
                        for h in range(HPC):
                            acc = ps_qk.tile([128, SC1], f32, tag="qk")
                            for c in range(n_din):
                                nc.tensor.matmul(
                                    acc,
                                    lhsT=w_s[:, c, h * HD : (h + 1) * HD],
                                    rhs=xt[:, c, :],
                                    start=(c == 0),
                                    stop=(c == n_din - 1),
                                )
                            raw = p1t.tile([128, SC1], f32r, tag="raw")
                            nc.scalar.copy(raw, acc)
                            rot = ps_rot.tile([128, SC1], f32, tag="rot")
                            nc.tensor.matmul(
                                rot, lhsT=rTs, rhs=raw, start=True, stop=True
                            )
                            t1 = p1t.tile([128, SC1], f32, tag="t1")
                            nc.vector.tensor_mul(t1, raw.bitcast(f32), cost)
                            t2 = p1t.tile([128, SC1], f32, tag="t2")
                            nc.vector.tensor_mul(t2, rot, sint)
                            nc.vector.tensor_add(store[:, h, s0 : s0 + SC1], t1, t2)

                    for sub in range(SC1 // 128):
                        vacc = ps_v.tile([128, DLOC], f32, tag="v")
                        for c in range(n_din):
                            nc.tensor.matmul(
                                vacc,
                                lhsT=xt[:, c, sub * 128 : (sub + 1) * 128],
                                rhs=wv_s[:, c, :],
                                start=(c == 0),
                                stop=(c == n_din - 1),
                            )
                        nc.scalar.copy(vS[:, s0 // 128 + sub, :], vacc)

            # ------------- phases 2+3: attention, then output projection -------------
            # pools for both phases coexist so phase-3 groups (per batch) can
            # start while later batches' attention is still running
            with (
                tc.tile_pool(name="persistB", bufs=1) as persistB,
                tc.tile_pool(name="p2", bufs=4) as p2,
                tc.tile_pool(name="p2l", bufs=3) as p2l,
                tc.tile_pool(name="p2r", bufs=2) as p2r,
                tc.tile_pool(name="p3", bufs=2) as p3,
                tc.tile_pool(name="ps_st", bufs=2, space="PSUM") as ps_st,
                tc.tile_pool(name="ps_o", bufs=2, space="PSUM") as ps_o,
                tc.tile_pool(name="ps3", bufs=4, space="PSUM") as ps3,
            ):
                uT = persistB.tile([128, HPC, bs], f32r)  # attn out, [d, h, b*s]

                SCG = min(2, s // QCH)
                n_scg_b = s // (SCG * QCH)  # phase-3 groups per batch

                def phase3_groups(bi):
                    # outT[dout, s] = woT.T @ uT for batch bi's s-range;
                    # emitted right after bi's attention so the PE queue
                    # pipelines projection bursts with attention tails
                    for oc in range(dim // 128):
                        o0 = oc * 128
                        for gl in range(n_scg_b):
                            g = bi * n_scg_b + gl
                            pos = [
                                ps3.tile([128, QCH], f32, tag="op", name=f"po{_j}")
                                for _j in range(SCG)
                            ]
                            for h in range(HPC):
                                for j in range(SCG):
                                    s0 = (g * SCG + j) * QCH
                                    nc.tensor.matmul(
                                        pos[j],
                                        lhsT=woT_s[:, h, o0 : o0 + 128],
                                        rhs=uT[:, h, s0 : s0 + QCH],
                                        start=(h == 0),
                                        stop=(h == HPC - 1),
                                    )
                            ot = p3.tile([128, SCG, QCH], f32, tag="ot")
                            for j in range(SCG):
                                if j % 2 == 0:
                                    nc.scalar.copy(ot[:, j, :], pos[j])
                                else:
                                    nc.vector.tensor_copy(ot[:, j, :], pos[j])
                            nc.sync.dma_start(
                                out=out_d[
                                    o0 : o0 + 128, g * SCG * QCH : (g + 1) * SCG * QCH
                                ],
                                in_=ot,
                            )

                for bi in range(b):
                    for h in range(HPC):
                        for qc in range(n_qc):
                            q0 = bi * s + qc * QCH
                            nkc = (qc + 1) * QCH // 128
                            outp = ps_o.tile([128, QCH], f32, tag="o")
                            lrep = ps_o.tile([128, QCH], f32, tag="o", name="lrep")
                            prev_pt = None
                            li = 0
                            for kc in range(nkc):
                                k0 = bi * s + kc * 128
                                st = ps_st.tile([128, QCH], f32, tag="st")
                                nc.tensor.matmul(
                                    st,
                                    lhsT=kT[:, h, k0 : k0 + 128],
                                    rhs=qT[:, h, q0 : q0 + QCH],
                                    start=True,
                                    stop=True,
                                )
                                di = kc - (nkc - 4)
                                if di >= 0:
                                    # additive -1e9 causal mask on raw scores
                                    nc.vector.tensor_add(st, st, masks_s[:, di, :])
                                pt = p2.tile([128, QCH], f32r, tag="pt")
                                nc.scalar.activation(pt, st, EXP, scale=scale)
                                nc.tensor.matmul(
                                    outp,
                                    lhsT=vS[
                                        :, bi * (s // 128) + kc, h * HD : (h + 1) * HD
                                    ],
                                    rhs=pt,
                                    start=(kc == 0),
                                    stop=(kc == nkc - 1),
                                )
                                if kc % 2 == 1:
                                    # softmax denominator: independent pair-sums
                                    # (DVE/GpSimd alternating), partition-reduced
                                    # by an interleaved ones-matmul accumulation
                                    lp = p2l.tile([128, QCH], f32r, tag="lp")
                                    eng = nc.vector if li % 2 == 0 else nc.gpsimd
                                    eng.tensor_add(lp, prev_pt, pt)
                                    nc.tensor.matmul(
                                        lrep,
                                        lhsT=ones,
                                        rhs=lp,
                                        start=(li == 0),
                                        stop=(li == nkc // 2 - 1),
                                    )
                                    li += 1
                                prev_pt = pt
                            rec = p2r.tile([128, QCH], f32, tag="rec")
                            nc.vector.reciprocal_approx_fast(rec, lrep)
                            nc.vector.tensor_mul(uT[:, h, q0 : q0 + QCH], outp, rec)
                    phase3_groups(bi)

    nc.compile()
    return nc


def make_in_maps(x, Wq, Wk, Wv, Wo, b=B, s=S, dim=DIM, n_cores=N_CORES):
    bs = b * s
    xT = np.ascontiguousarray(x.reshape(bs, dim).T.astype(np.float32))
    cosT1, sinT1 = _rope_tables_T(s, HD)
    cosT = np.ascontiguousarray(np.tile(cosT1, (1, b)))
    sinT = np.ascontiguousarray(np.tile(sinT1, (1, b)))
    rT = _rot_matrix_T(HD)
    ones = np.ones((HD, HD), dtype=np.float32)
    import ml_dtypes
    masks = _causal_masks(QCH).astype(ml_dtypes.bfloat16)
    in_maps = []
    for c in range(n_cores):
        sl = slice(c * DLOC, (c + 1) * DLOC)
        in_maps.append(
            {
                "xT": xT,
                "wqT": np.ascontiguousarray(Wq[sl, :].T.astype(np.float32)),
                "wkT": np.ascontiguousarray(Wk[sl, :].T.astype(np.float32)),
                "wvT": np.ascontiguousarray(Wv[sl, :].T.astype(np.float32)),
                "woT": np.ascontiguousarray(Wo[:, sl].T.astype(np.float32)),
                "cosT": cosT,
                "sinT": sinT,
                "rT": rT,
                "ones": ones,
                "masks": masks,
            }
        )
    return in_maps


def kernel(x, Wq, Wk, Wv, Wo, _trace=False):
    """Full-input / full-output entry point. Shards over 8 cores internally."""
    if "/opt/trn_rl_repo" not in sys.path:
        sys.path.insert(0, "/opt/trn_rl_repo")
    from concourse.bass_utils import run_bass_kernel_spmd

    x = np.asarray(x, dtype=np.float32)
    Wq, Wk, Wv, Wo = (np.asarray(w, dtype=np.float32) for w in (Wq, Wk, Wv, Wo))

    key = (B, S, DIM)
    if key not in _PROGRAM_CACHE:
        _PROGRAM_CACHE[key] = build_program(B, S, DIM)
    nc = _PROGRAM_CACHE[key]

    in_maps = make_in_maps(x, Wq, Wk, Wv, Wo)
    res = run_bass_kernel_spmd(
        nc, in_maps, core_ids=list(range(N_CORES)), trace=_trace
    )
    kernel.last_results = res
    acc = res.results[0]["out"].astype(np.float32)
    for c in range(1, N_CORES):
        acc = acc + res.results[c]["out"]
    return np.ascontiguousarray(acc.T).reshape(B, S, DIM)



# revision 3
# speedup vs baseline: 1.2390x; 1.2390x over previous
"""Multi-head causal self-attention with RoPE, tensor-parallel over heads
across 8 Trainium2 NeuronCores.

Strategy (Megatron-style TP over heads), v2:
  - Each core owns 2 of the 16 heads: rows [c*256,(c+1)*256) of Wq/Wk/Wv
    and the matching columns of Wo. Host sums the 8 partial outputs.
  - All matmul operands in bf16 (PSUM accumulation stays fp32); the
    rel-err budget (2e-2) dwarfs bf16 noise (~5e-3).
  - Batch-pipelined emission: p1(b0) -> [p1(b1) || attn(b0)] ->
    [attn(b1) || p3(b0)] -> p3(b1). Phases are Python generators that
    yield between instruction bundles; a round-robin scheduler
    interleaves their emission so independent projection matmuls fill
    the in-order PE queue between each score-matmul -> exp -> AV-matmul
    dependency chain (the v1 bottleneck: PE idled ~400ns per k-chunk
    waiting on the Scalar-engine exp).
  - Causal handling: off-diagonal k-chunks run full-width; the 4
    diagonal chunks of each 512-wide q-chunk run with a reduced column
    range plus a 128x128 additive triangle mask.
  - Startup: critical DMAs (wq group 0, first x chunk) issue first,
    split across the sync and scalar HWDGE queues; phase-2/3 constants
    issue later / on gpsimd SWDGE.
"""

import sys

import numpy as np

B, S, DIM = 2, 2048, 2048
NUM_HEADS = 16
HD = 128
N_CORES = 8
HPC = NUM_HEADS // N_CORES  # heads per core
DLOC = HPC * HD             # per-core slice of the model dim
ROPE_BASE = 10000.0
NDIN = DIM // 128           # contraction chunks for projections
SC1 = 512                   # phase-1 s-chunk
QCH = 512                   # attention q-chunk
NQC = S // QCH              # q-chunks per batch
NS1B = S // SC1             # phase-1 s-chunks per batch

_PROGRAM_CACHE = {}


def _rope_tables_T(seq_len, head_dim):
    # match reference float32 arithmetic: inv_freq over even indices,
    # emb = cat(freqs, freqs); returned transposed [head_dim, seq_len]
    inv_freq = (
        1.0
        / (np.float32(ROPE_BASE)
           ** (np.arange(0, head_dim, 2, dtype=np.float32) / np.float32(head_dim)))
    ).astype(np.float32)
    t = np.arange(seq_len, dtype=np.float32)
    freqs = np.outer(t, inv_freq).astype(np.float32)      # [S, D/2]
    emb = np.concatenate([freqs, freqs], axis=-1)         # [S, D]
    return (
        np.ascontiguousarray(np.cos(emb).astype(np.float32).T),
        np.ascontiguousarray(np.sin(emb).astype(np.float32).T),
    )


def _rot_matrix_T(head_dim):
    # rotated = cat(-x[1::2], x[::2]) = R @ x; return R.T [D, D]
    d2 = head_dim // 2
    R = np.zeros((head_dim, head_dim), dtype=np.float32)
    for dp in range(d2):
        R[dp, 2 * dp + 1] = -1.0
    for dp in range(d2, head_dim):
        R[dp, 2 * (dp - d2)] = 1.0
    return np.ascontiguousarray(R.T)


def _tri_mask():
    # tri[kk, qq] = 0 if kk <= qq else -1e9 (additive, pre-exp-scale)
    kk = np.arange(128)[:, None]
    qq = np.arange(128)[None, :]
    return np.where(kk <= qq, 0.0, -1e9).astype(np.float32)


def _run_interleaved(*specs):
    """specs: (generator, steps_per_round). Round-robin until exhausted."""
    live = [[iter(g), k] for g, k in specs]
    while live:
        for item in list(live):
            g, k = item
            try:
                for _ in range(k):
                    next(g)
            except StopIteration:
                live.remove(item)


def build_program(b=B, s=S, dim=DIM):
    """Builds the per-core SPMD Bass program (identical on every core)."""
    if "/opt/trn_rl_repo" not in sys.path:
        sys.path.insert(0, "/opt/trn_rl_repo")
    import concourse.bacc as bacc
    import concourse.mybir as mybir
    import concourse.tile as tile

    f32 = mybir.dt.float32
    bf16 = mybir.dt.bfloat16
    EXP = mybir.ActivationFunctionType.Exp

    bs = b * s
    scale = float(HD) ** -0.5
    nsc = s // 128  # 128-token chunks per batch

    nc = bacc.Bacc("TRN2", target_bir_lowering=False, debug=False)

    xT_d = nc.dram_tensor("xT", [dim, bs], bf16, kind="ExternalInput")
    wqT_d = nc.dram_tensor("wqT", [dim, DLOC], bf16, kind="ExternalInput")
    wkT_d = nc.dram_tensor("wkT", [dim, DLOC], bf16, kind="ExternalInput")
    wvT_d = nc.dram_tensor("wvT", [dim, DLOC], bf16, kind="ExternalInput")
    woT_d = nc.dram_tensor("woT", [DLOC, dim], bf16, kind="ExternalInput")
    cosT_d = nc.dram_tensor("cosT", [HD, bs], bf16, kind="ExternalInput")
    sinT_d = nc.dram_tensor("sinT", [HD, bs], bf16, kind="ExternalInput")
    rT_d = nc.dram_tensor("rT", [HD, HD], bf16, kind="ExternalInput")
    ones_d = nc.dram_tensor("ones", [HD, HD], bf16, kind="ExternalInput")
    tri_d = nc.dram_tensor("tri", [HD, HD], bf16, kind="ExternalInput")
    out_d = nc.dram_tensor("out", [dim, bs], bf16, kind="ExternalOutput")

    with tile.TileContext(nc) as tc:
        with (
            tc.tile_pool(name="persist", bufs=1) as persist,
            tc.tile_pool(name="work", bufs=2) as work,
            tc.tile_pool(name="ptp", bufs=4) as ptp,
            tc.tile_pool(name="otp", bufs=2) as otp,
            tc.tile_pool(name="ps_at", bufs=1, space="PSUM") as ps_at,
        ):
            # ---- persistent tiles ----
            qT = persist.tile([128, HPC, bs], bf16)   # roped q, [d, h, tok]
            kT = persist.tile([128, HPC, bs], bf16)
            vS = persist.tile([128, bs // 128, DLOC], bf16)  # [tok, chunk, d]
            uT = persist.tile([128, HPC, bs], bf16)   # attention out
            wq_s = persist.tile([128, NDIN, DLOC], bf16)
            wk_s = persist.tile([128, NDIN, DLOC], bf16)
            wv_s = persist.tile([128, NDIN, DLOC], bf16)
            woT_s = persist.tile([128, HPC, dim], bf16)
            cosS = persist.tile([128, bs], bf16)
            sinS = persist.tile([128, bs], bf16)
            rTs = persist.tile([HD, HD], bf16)
            onesS = persist.tile([HD, HD], bf16)
            triS = persist.tile([HD, HD], bf16)

            # ---- startup DMAs, critical first, split across queues ----
            GW = 4  # weight chunk-group size
            def wsrc(wd):
                return wd.rearrange("(c p) m -> p c m", p=128)

            nc.sync.dma_start(out=wq_s[:, 0:GW, :], in_=wsrc(wqT_d)[:, 0:GW, :])
            nc.scalar.dma_start(out=wk_s[:, 0:GW, :], in_=wsrc(wkT_d)[:, 0:GW, :])

            def startup_rest():
                nc.scalar.dma_start(out=cosS, in_=cosT_d[:])
                nc.scalar.dma_start(out=sinS, in_=sinT_d[:])
                nc.gpsimd.dma_start(out=rTs, in_=rT_d[:])
                nc.gpsimd.dma_start(out=onesS, in_=ones_d[:])
                nc.gpsimd.dma_start(out=triS, in_=tri_d[:])
                nc.scalar.dma_start(out=wv_s[:, 0:GW, :], in_=wsrc(wvT_d)[:, 0:GW, :])
                for g0 in range(GW, NDIN, GW):
                    nc.sync.dma_start(
                        out=wq_s[:, g0:g0 + GW, :], in_=wsrc(wqT_d)[:, g0:g0 + GW, :]
                    )
                    nc.scalar.dma_start(
                        out=wk_s[:, g0:g0 + GW, :], in_=wsrc(wkT_d)[:, g0:g0 + GW, :]
                    )
                    nc.scalar.dma_start(
                        out=wv_s[:, g0:g0 + GW, :], in_=wsrc(wvT_d)[:, g0:g0 + GW, :]
                    )
                # phase-3 weights: needed only after attn(b0); issue last
                nc.scalar.dma_start(
                    out=woT_s, in_=woT_d.rearrange("(h p) n -> p h n", p=128)
                )

            with (
                tc.tile_pool(name="p1x", bufs=2) as p1x,
                tc.tile_pool(name="p1ps", bufs=1, space="PSUM") as p1ps,
            ):
                # ------------- phase 1: qkv projections + RoPE -------------
                def p1_gen(bi, first=False):
                    for si in range(NS1B):
                        s0 = bi * s + si * SC1
                        xt = p1x.tile([128, NDIN, SC1], bf16, tag="xt")
                        xsrc = xT_d[:, s0:s0 + SC1].rearrange(
                            "(c p) t -> p c t", p=128
                        )
                        nh = NDIN // 2
                        nc.sync.dma_start(out=xt[:, :nh, :], in_=xsrc[:, :nh, :])
                        nc.sync.dma_start(out=xt[:, nh:, :], in_=xsrc[:, nh:, :])
                        if first and si == 0:
                            startup_rest()
                        yield
                        # interleave v sub-chains between q/k chains so the
                        # single v PSUM buf drains while qk chains run
                        vq = []
                        for sub in range(SC1 // 128):
                            vq.append(sub)

                        def v_chain(sub):
                            vacc = p1ps.tile([128, DLOC], f32, tag="v", bufs=1)
                            for c in range(NDIN):
                                nc.tensor.matmul(
                                    vacc,
                                    lhsT=xt[:, c, sub * 128:(sub + 1) * 128],
                                    rhs=wv_s[:, c, :],
                                    start=(c == 0),
                                    stop=(c == NDIN - 1),
                                )
                                if c % 4 == 3:
                                    yield
                            nc.scalar.copy(vS[:, s0 // 128 + sub, :], vacc)
                            yield

                        for w_t, store in ((wq_s, qT), (wk_s, kT)):
                            for h in range(HPC):
                                acc = p1ps.tile([128, SC1], f32, tag="qk", bufs=2)
                                for c in range(NDIN):
                                    nc.tensor.matmul(
                                        acc,
                                        lhsT=w_t[:, c, h * HD:(h + 1) * HD],
                                        rhs=xt[:, c, :],
                                        start=(c == 0),
                                        stop=(c == NDIN - 1),
                                    )
                                    if c % 4 == 3:
                                        yield
                                raw = p1x.tile([128, SC1], bf16, tag="raw")
                                nc.vector.tensor_copy(raw, acc)
                                rot = p1ps.tile([128, SC1], f32, tag="rot", bufs=1)
                                nc.tensor.matmul(
                                    rot, lhsT=rTs, rhs=raw, start=True, stop=True
                                )
                                yield
                                t1 = p1x.tile([128, SC1], bf16, tag="t1")
                                nc.vector.tensor_mul(t1, raw, cosS[:, s0:s0 + SC1])
                                t2 = p1x.tile([128, SC1], bf16, tag="t2")
                                nc.vector.tensor_mul(t2, rot, sinS[:, s0:s0 + SC1])
                                nc.vector.tensor_add(
                                    store[:, h, s0:s0 + SC1], t1, t2
                                )
                                yield
                                if vq:
                                    yield from v_chain(vq.pop(0))
                        while vq:
                            yield from v_chain(vq.pop(0))

                # ------------- phase 2: causal attention -------------
                def attn_gen(bi):
                    for h in range(HPC):
                        for qc in range(NQC):
                            q0 = bi * s + qc * QCH
                            nkc = (qc + 1) * QCH // 128
                            outp = ps_at.tile([128, QCH], f32, tag="o", bufs=1)
                            lrep = ps_at.tile([128, QCH], f32, tag="l", bufs=1)
                            npair = (nkc - 4) // 2
                            nones = npair + 4
                            li = 0
                            prev_pt = None
                            for kc in range(nkc):
                                k0 = bi * s + kc * 128
                                di = kc - (nkc - 4)
                                lo = max(0, di * 128)
                                st = ps_at.tile([128, QCH], f32, tag="st", bufs=2)
                                nc.tensor.matmul(
                                    st[:, lo:],
                                    lhsT=kT[:, h, k0:k0 + 128],
                                    rhs=qT[:, h, q0 + lo:q0 + QCH],
                                    start=True,
                                    stop=True,
                                )
                                if di >= 0:
                                    nc.vector.tensor_add(
                                        st[:, lo:lo + 128], st[:, lo:lo + 128], triS
                                    )
                                pt = ptp.tile([128, QCH], bf16, tag="pt")
                                nc.scalar.activation(
                                    pt[:, lo:], st[:, lo:], EXP, scale=scale
                                )
                                yield
                                nc.tensor.matmul(
                                    outp[:, lo:],
                                    lhsT=vS[:, bi * nsc + kc, h * HD:(h + 1) * HD],
                                    rhs=pt[:, lo:],
                                    start=(kc == 0),
                                    stop=(kc == nkc - 1),
                                )
                                if di >= 0:
                                    # diagonal chunks: single range-reduced
                                    # denominator matmul
                                    nc.tensor.matmul(
                                        lrep[:, lo:],
                                        lhsT=onesS,
                                        rhs=pt[:, lo:],
                                        start=(li == 0),
                                        stop=(li == nones - 1),
                                    )
                                    li += 1
                                elif kc % 2 == 1:
                                    # off-diagonal: pair-sum on DVE/GpSimd,
                                    # then one denominator matmul per pair
                                    lp = work.tile([128, QCH], bf16, tag="lp")
                                    eng = nc.vector if li % 2 == 0 else nc.gpsimd
                                    eng.tensor_add(lp, prev_pt, pt)
                                    nc.tensor.matmul(
                                        lrep,
                                        lhsT=onesS,
                                        rhs=lp,
                                        start=(li == 0),
                                        stop=(li == nones - 1),
                                    )
                                    li += 1
                                prev_pt = pt
                                yield
                            rec = work.tile([128, QCH], f32, tag="rec")
                            nc.vector.reciprocal_approx_fast(rec, lrep)
                            nc.vector.tensor_mul(
                                uT[:, h, q0:q0 + QCH], outp, rec
                            )
                            yield

                # seg1: projections for batch 0 alone
                _run_interleaved((p1_gen(0, first=True), 1))
                # seg2: projections(b1) interleaved with attention(b0)
                _run_interleaved((p1_gen(1), 1), (attn_gen(0), 1))

            # p1 PSUM pools closed; banks free for phase 3
            with tc.tile_pool(name="ps3", bufs=2, space="PSUM") as ps3:
                # ------------- phase 3: output projection -------------
                def p3_gen(bi):
                    copy_engs = (
                        (nc.vector, nc.vector) if bi == 0
                        else (nc.scalar, nc.vector)
                    )
                    for wave in range(s // 1024):
                        c0 = bi * s + wave * 1024
                        for oc in range(dim // 128):
                            o0 = oc * 128
                            ot = otp.tile([128, 1024], bf16, tag="ot")
                            for j in range(2):
                                pos = ps3.tile([128, 512], f32, tag="p3")
                                for h in range(HPC):
                                    nc.tensor.matmul(
                                        pos,
                                        lhsT=woT_s[:, h, o0:o0 + 128],
                                        rhs=uT[:, h, c0 + j * 512:c0 + (j + 1) * 512],
                                        start=(h == 0),
                                        stop=(h == HPC - 1),
                                    )
                                eng = copy_engs[j]
                                if eng is nc.vector:
                                    eng.tensor_copy(ot[:, j * 512:(j + 1) * 512], pos)
                                else:
                                    eng.copy(ot[:, j * 512:(j + 1) * 512], pos)
                                yield
                            nc.sync.dma_start(
                                out=out_d[o0:o0 + 128, c0:c0 + 1024], in_=ot
                            )
                            yield

                # seg3: attention(b1) interleaved with output proj(b0)
                _run_interleaved((attn_gen(1), 2), (p3_gen(0), 1))
                # seg4: output projection for batch 1
                _run_interleaved((p3_gen(1), 1))

    nc.compile()
    return nc


def make_in_maps(x, Wq, Wk, Wv, Wo, b=B, s=S, dim=DIM, n_cores=N_CORES):
    import ml_dtypes

    bf = ml_dtypes.bfloat16
    bs = b * s
    xT = np.ascontiguousarray(x.reshape(bs, dim).T).astype(bf)
    cosT1, sinT1 = _rope_tables_T(s, HD)
    cosT = np.ascontiguousarray(np.tile(cosT1, (1, b))).astype(bf)
    sinT = np.ascontiguousarray(np.tile(sinT1, (1, b))).astype(bf)
    rT = _rot_matrix_T(HD).astype(bf)
    ones = np.ones((HD, HD), dtype=np.float32).astype(bf)
    tri = _tri_mask().astype(bf)
    in_maps = []
    for c in range(n_cores):
        sl = slice(c * DLOC, (c + 1) * DLOC)
        in_maps.append(
            {
                "xT": xT,
                "wqT": np.ascontiguousarray(Wq[sl, :].T).astype(bf),
                "wkT": np.ascontiguousarray(Wk[sl, :].T).astype(bf),
                "wvT": np.ascontiguousarray(Wv[sl, :].T).astype(bf),
                "woT": np.ascontiguousarray(Wo[:, sl].T).astype(bf),
                "cosT": cosT,
                "sinT": sinT,
                "rT": rT,
                "ones": ones,
                "tri": tri,
            }
        )
    return in_maps


def kernel(x, Wq, Wk, Wv, Wo, _trace=False):
    """Full-input / full-output entry point. Shards over 8 cores internally."""
    if "/opt/trn_rl_repo" not in sys.path:
        sys.path.insert(0, "/opt/trn_rl_repo")
    from concourse.bass_utils import run_bass_kernel_spmd

    x = np.asarray(x, dtype=np.float32)
    Wq, Wk, Wv, Wo = (np.asarray(w, dtype=np.float32) for w in (Wq, Wk, Wv, Wo))

    key = (B, S, DIM)
    if key not in _PROGRAM_CACHE:
        _PROGRAM_CACHE[key] = build_program(B, S, DIM)
    nc = _PROGRAM_CACHE[key]

    in_maps = make_in_maps(x, Wq, Wk, Wv, Wo)
    res = run_bass_kernel_spmd(
        nc, in_maps, core_ids=list(range(N_CORES)), trace=_trace
    )
    kernel.last_results = res
    acc = res.results[0]["out"].astype(np.float32)
    for c in range(1, N_CORES):
        acc = acc + res.results[c]["out"].astype(np.float32)
    return np.ascontiguousarray(acc.T).reshape(B, S, DIM)


# revision 20
# speedup vs baseline: 1.5404x; 1.2433x over previous
"""Multi-head causal self-attention with RoPE, tensor-parallel over heads
across 8 Trainium2 NeuronCores.

Strategy (Megatron-style TP over heads), v4:
  - Each core owns 2 of the 16 heads: rows [c*256,(c+1)*256) of Wq/Wk/Wv
    and the matching columns of Wo. Host sums the 8 partial outputs.
  - q/k projections run in fp8e4m3 with DoubleRow perf mode (0.5
    cycles/row, 2x PE throughput); the host pre-scales Wq/Wk by 64 to
    center the fp8 dynamic range, and the 64*64 factor is folded into
    the exp scale. The q/k quantization noise washes out through the
    softmax (diffuse attention); the value path (v, Wo) stays bf16.
  - Everything else in bf16 (PSUM accumulation fp32).
  - All DRAM operands are host-prearranged so every DMA is contiguous
    per partition (128 descriptors, cheap HWDGE issue); out is written
    oc-major [16, 128, bs] and reassembled on host.
  - Batch-pipelined generator emission: p1(b0) -> [p1(b1) || attn(b0)]
    -> [attn(b1) || p3(ready slabs)] -> p3(last slab). Independent
    matmuls fill the in-order PE queue between each score-matmul ->
    exp -> AV-matmul dependency chain.
  - Causal: off-diagonal k-chunks full-width; the 4 diagonal chunks of
    each 512-wide q-chunk use a reduced column range plus a 128x128
    additive triangle mask.
"""

import sys

import numpy as np

B, S, DIM = 2, 2048, 2048
NUM_HEADS = 16
HD = 128
N_CORES = 8
HPC = NUM_HEADS // N_CORES  # heads per core
DLOC = HPC * HD             # per-core slice of the model dim
ROPE_BASE = 10000.0
NDIN = DIM // 128           # contraction chunks for projections
SC1 = 512                   # phase-1 s-chunk
QCH = 512                   # attention q-chunk
NQC = S // QCH              # q-chunks per batch
NS1B = S // SC1             # phase-1 s-chunks per batch
WSCALE = 64.0               # fp8 pre-scale on Wq/Wk

_PROGRAM_CACHE = {}


def _rope_tables_T(seq_len, head_dim):
    # match reference float32 arithmetic: inv_freq over even indices,
    # emb = cat(freqs, freqs); returned transposed [head_dim, seq_len]
    inv_freq = (
        1.0
        / (np.float32(ROPE_BASE)
           ** (np.arange(0, head_dim, 2, dtype=np.float32) / np.float32(head_dim)))
    ).astype(np.float32)
    t = np.arange(seq_len, dtype=np.float32)
    freqs = np.outer(t, inv_freq).astype(np.float32)      # [S, D/2]
    emb = np.concatenate([freqs, freqs], axis=-1)         # [S, D]
    return (
        np.ascontiguousarray(np.cos(emb).astype(np.float32).T),
        np.ascontiguousarray(np.sin(emb).astype(np.float32).T),
    )


def _rot_matrix_T(head_dim):
    # rotated = cat(-x[1::2], x[::2]) = R @ x; return R.T [D, D]
    d2 = head_dim // 2
    R = np.zeros((head_dim, head_dim), dtype=np.float32)
    for dp in range(d2):
        R[dp, 2 * dp + 1] = -1.0
    for dp in range(d2, head_dim):
        R[dp, 2 * (dp - d2)] = 1.0
    return np.ascontiguousarray(R.T)


def _tri_mask():
    # tri[kk, qq] = 0 if kk <= qq else -1e9 (additive, pre-exp-scale)
    kk = np.arange(128)[:, None]
    qq = np.arange(128)[None, :]
    return np.where(kk <= qq, 0.0, -1e9).astype(np.float32)


def _run_interleaved(*specs):
    """specs: (generator, steps_per_round). Round-robin until exhausted."""
    live = [[iter(g), k] for g, k in specs]
    while live:
        for item in list(live):
            g, k = item
            try:
                for _ in range(k):
                    next(g)
            except StopIteration:
                live.remove(item)


def build_program(b=B, s=S, dim=DIM):
    """Builds the per-core SPMD Bass program (identical on every core)."""
    if "/opt/trn_rl_repo" not in sys.path:
        sys.path.insert(0, "/opt/trn_rl_repo")
    import concourse.bacc as bacc
    import concourse.mybir as mybir
    import concourse.tile as tile

    f32 = mybir.dt.float32
    bf16 = mybir.dt.bfloat16
    fp8 = mybir.dt.float8e4
    EXP = mybir.ActivationFunctionType.Exp
    DR = mybir.MatmulPerfMode.DoubleRow

    bs = b * s
    scale = float(HD) ** -0.5 / (WSCALE * WSCALE)
    nsc = s // 128   # 128-token chunks per batch
    ngc = bs // SC1  # global 512-token chunks

    nc = bacc.Bacc("TRN2", target_bir_lowering=False, debug=False)

    # host-prearranged layouts: contiguous per partition
    xP_d = nc.dram_tensor("xP", [ngc, 128, NDIN * SC1], bf16, kind="ExternalInput")
    x8P_d = nc.dram_tensor("x8P", [ngc, 128, NDIN * SC1], fp8, kind="ExternalInput")
    wq8_d = nc.dram_tensor("wq8", [128, NDIN * DLOC], fp8, kind="ExternalInput")
    wk8_d = nc.dram_tensor("wk8", [128, NDIN * DLOC], fp8, kind="ExternalInput")
    wqb_d = nc.dram_tensor("wqb", [128, NDIN * DLOC], bf16, kind="ExternalInput")
    wkb_d = nc.dram_tensor("wkb", [128, NDIN * DLOC], bf16, kind="ExternalInput")
    wvP_d = nc.dram_tensor("wvP", [128, NDIN * DLOC], bf16, kind="ExternalInput")
    woP_d = nc.dram_tensor("woP", [128, HPC * dim], bf16, kind="ExternalInput")
    cosT_d = nc.dram_tensor("cosT", [HD, bs], bf16, kind="ExternalInput")
    sinT_d = nc.dram_tensor("sinT", [HD, bs], bf16, kind="ExternalInput")
    rT_d = nc.dram_tensor("rT", [HD, HD], bf16, kind="ExternalInput")
    ones_d = nc.dram_tensor("ones", [HD, HD], bf16, kind="ExternalInput")
    tri_d = nc.dram_tensor("tri", [HD, HD], bf16, kind="ExternalInput")
    out_d = nc.dram_tensor("out", [dim // 128, 128, bs], bf16, kind="ExternalOutput")

    with tile.TileContext(nc) as tc:
        with (
            tc.tile_pool(name="persist", bufs=1) as persist,
            tc.tile_pool(name="work", bufs=2) as work,
            tc.tile_pool(name="ptp", bufs=4) as ptp,
            tc.tile_pool(name="otp", bufs=3) as otp,
            tc.tile_pool(name="ps_at", bufs=1, space="PSUM") as ps_at,
        ):
            # ---- persistent tiles ----
            qT = persist.tile([128, HPC, bs], bf16)   # roped q (x64), [d, h, tok]
            kT = persist.tile([128, HPC, bs], bf16)
            vS = persist.tile([128, bs // 128, DLOC], bf16)  # [tok, chunk, d]
            uT = persist.tile([128, HPC, bs], bf16)   # attention out
            wq_s = persist.tile([128, NDIN, DLOC], fp8)
            wk_s = persist.tile([128, NDIN, DLOC], fp8)
            # bf16 copies for the first 512 tokens of each batch: early
            # causal rows have concentrated attention, so fp8 q/k noise
            # doesn't wash out there (x64-scaled like the fp8 path)
            wqb_s = persist.tile([128, NDIN, DLOC], bf16)
            wkb_s = persist.tile([128, NDIN, DLOC], bf16)
            wv_s = persist.tile([128, NDIN, DLOC], bf16)
            woT_s = persist.tile([128, HPC, dim], bf16)
            cosS = persist.tile([128, bs], bf16)
            sinS = persist.tile([128, bs], bf16)
            rTs = persist.tile([HD, HD], bf16)
            onesS = persist.tile([HD, HD], bf16)
            triS = persist.tile([HD, HD], bf16)

            # ---- startup DMAs: critical first, split across queues ----
            # first chain (si=0, bf16 path) needs wqb + xt interleaved in
            # consumption order on sync; everything else on scalar/gpsimd
            def wview(wd):
                return wd.rearrange("p (c m) -> p c m", c=NDIN)

            nc.sync.dma_start(out=wqb_s[:, 0:8, :], in_=wview(wqb_d)[:, 0:8, :])
            nc.scalar.dma_start(out=wkb_s, in_=wview(wkb_d))
            nc.gpsimd.dma_start(out=rTs, in_=rT_d[:])
            nc.gpsimd.dma_start(out=onesS, in_=ones_d[:])
            nc.gpsimd.dma_start(out=triS, in_=tri_d[:])

            def startup_x0(xt, xsrc):
                nc.sync.dma_start(out=xt[:, 0:4, :], in_=xsrc[:, 0:4, :])
                nc.sync.dma_start(out=wqb_s[:, 8:16, :], in_=wview(wqb_d)[:, 8:16, :])
                nc.sync.dma_start(out=xt[:, 4:8, :], in_=xsrc[:, 4:8, :])
                nc.sync.dma_start(out=xt[:, 8:12, :], in_=xsrc[:, 8:12, :])
                nc.sync.dma_start(out=xt[:, 12:16, :], in_=xsrc[:, 12:16, :])

            def startup_rest():
                nc.scalar.dma_start(out=cosS, in_=cosT_d[:])
                nc.scalar.dma_start(out=sinS, in_=sinT_d[:])
                nc.scalar.dma_start(out=wv_s, in_=wview(wvP_d))
                nc.scalar.dma_start(out=wq_s, in_=wview(wq8_d))
                nc.scalar.dma_start(out=wk_s, in_=wview(wk8_d))
                # phase-3 weights: needed only after attn(b0); issue last
                nc.scalar.dma_start(
                    out=woT_s, in_=woP_d.rearrange("p (h n) -> p h n", h=HPC)
                )

            with (
                tc.tile_pool(name="p1x", bufs=2) as p1x,
                tc.tile_pool(name="p1ps", bufs=1, space="PSUM") as p1ps,
            ):
                # ------------- phase 1: qkv projections + RoPE -------------
                def p1_gen(bi, first=False):
                    for si in range(NS1B):
                        s0 = bi * s + si * SC1
                        g = s0 // SC1
                        xt = p1x.tile([128, NDIN, SC1], bf16, tag="xt")
                        xsrc = xP_d[g].rearrange("p (c t) -> p c t", c=NDIN)
                        if si == 0:
                            # bf16 q/k path for the first 512 tokens
                            xt8 = None
                            if first:
                                startup_x0(xt, xsrc)
                                startup_rest()
                            else:
                                nc.sync.dma_start(out=xt[:, :8, :], in_=xsrc[:, :8, :])
                                nc.sync.dma_start(out=xt[:, 8:, :], in_=xsrc[:, 8:, :])
                        else:
                            xt8 = p1x.tile([128, NDIN, SC1], fp8, tag="xt8")
                            nc.sync.dma_start(
                                out=xt8,
                                in_=x8P_d[g].rearrange("p (c t) -> p c t", c=NDIN),
                            )
                            nc.sync.dma_start(out=xt[:, :8, :], in_=xsrc[:, :8, :])
                            nc.sync.dma_start(out=xt[:, 8:, :], in_=xsrc[:, 8:, :])
                        yield
                        vq = [0, 1, 2, 3]

                        def v_chain(sub):
                            vacc = p1ps.tile([128, DLOC], f32, tag="v", bufs=1)
                            for c in range(NDIN):
                                nc.tensor.matmul(
                                    vacc,
                                    lhsT=xt[:, c, sub * 128:(sub + 1) * 128],
                                    rhs=wv_s[:, c, :],
                                    start=(c == 0),
                                    stop=(c == NDIN - 1),
                                )
                                if c % 4 == 3:
                                    yield
                            nc.scalar.copy(vS[:, s0 // 128 + sub, :], vacc)
                            yield

                        ci = 0
                        wpairs = (
                            ((wqb_s, qT), (wkb_s, kT)) if si == 0
                            else ((wq_s, qT), (wk_s, kT))
                        )
                        for w_t, store in wpairs:
                            for h in range(HPC):
                                acc = p1ps.tile([128, SC1], f32, tag="qk", bufs=2)
                                if si == 0:
                                    for c in range(NDIN):
                                        nc.tensor.matmul(
                                            acc,
                                            lhsT=w_t[:, c, h * HD:(h + 1) * HD],
                                            rhs=xt[:, c, :],
                                            start=(c == 0),
                                            stop=(c == NDIN - 1),
                                        )
                                        if c % 4 == 3:
                                            yield
                                else:
                                    for c2 in range(0, NDIN, 2):
                                        nc.tensor.matmul(
                                            acc,
                                            lhsT=w_t[:, c2:c2 + 2, h * HD:(h + 1) * HD],
                                            rhs=xt8[:, c2:c2 + 2, :],
                                            start=(c2 == 0),
                                            stop=(c2 == NDIN - 2),
                                            perf_mode=DR,
                                        )
                                        if c2 % 4 == 2:
                                            yield
                                raw = p1x.tile([128, SC1], bf16, tag="raw")
                                nc.vector.tensor_copy(raw, acc)
                                rot = p1ps.tile([128, SC1], f32, tag="rot", bufs=1)
                                nc.tensor.matmul(
                                    rot, lhsT=rTs, rhs=raw, start=True, stop=True
                                )
                                yield
                                t1 = p1x.tile([128, SC1], bf16, tag="t1")
                                nc.vector.tensor_mul(t1, raw, cosS[:, s0:s0 + SC1])
                                t2 = p1x.tile([128, SC1], bf16, tag="t2")
                                nc.vector.tensor_mul(t2, rot, sinS[:, s0:s0 + SC1])
                                nc.vector.tensor_add(
                                    store[:, h, s0:s0 + SC1], t1, t2
                                )
                                yield
                                ci += 1
                                # delay v chains past the first two qk chains
                                # (wv lands late in the startup order)
                                if vq and ci >= 2:
                                    yield from v_chain(vq.pop(0))
                        while vq:
                            yield from v_chain(vq.pop(0))

                # ------------- phase 2: causal attention -------------
                # p3q: output-projection slabs whose uT columns are complete
                p3q = []

                def attn_gen(bi):
                    for qc in range(NQC):
                        for h in range(HPC):
                            q0 = bi * s + qc * QCH
                            nkc = (qc + 1) * QCH // 128
                            outp = ps_at.tile([128, QCH], f32, tag="o", bufs=1)
                            lrep = ps_at.tile([128, QCH], f32, tag="l", bufs=1)
                            npair = (nkc - 4) // 2
                            nones = npair + 4
                            li = 0
                            prev_pt = None
                            for kc in range(nkc):
                                k0 = bi * s + kc * 128
                                di = kc - (nkc - 4)
                                lo = max(0, di * 128)
                                st = ps_at.tile([128, QCH], f32, tag="st", bufs=2)
                                nc.tensor.matmul(
                                    st[:, lo:],
                                    lhsT=kT[:, h, k0:k0 + 128],
                                    rhs=qT[:, h, q0 + lo:q0 + QCH],
                                    start=True,
                                    stop=True,
                                )
                                if di >= 0:
                                    nc.vector.tensor_add(
                                        st[:, lo:lo + 128], st[:, lo:lo + 128], triS
                                    )
                                pt = ptp.tile([128, QCH], bf16, tag="pt")
                                nc.scalar.activation(
                                    pt[:, lo:], st[:, lo:], EXP, scale=scale
                                )
                                yield
                                nc.tensor.matmul(
                                    outp[:, lo:],
                                    lhsT=vS[:, bi * nsc + kc, h * HD:(h + 1) * HD],
                                    rhs=pt[:, lo:],
                                    start=(kc == 0),
                                    stop=(kc == nkc - 1),
                                )
                                if di >= 0:
                                    # diagonal chunks: single range-reduced
                                    # denominator matmul
                                    nc.tensor.matmul(
                                        lrep[:, lo:],
                                        lhsT=onesS,
                                        rhs=pt[:, lo:],
                                        start=(li == 0),
                                        stop=(li == nones - 1),
                                    )
                                    li += 1
                                elif kc % 2 == 1:
                                    # off-diagonal: pair-sum on DVE/GpSimd,
                                    # then one denominator matmul per pair
                                    lp = work.tile([128, QCH], bf16, tag="lp")
                                    eng = nc.vector if li % 2 == 0 else nc.gpsimd
                                    eng.tensor_add(lp, prev_pt, pt)
                                    nc.tensor.matmul(
                                        lrep,
                                        lhsT=onesS,
                                        rhs=lp,
                                        start=(li == 0),
                                        stop=(li == nones - 1),
                                    )
                                    li += 1
                                prev_pt = pt
                                yield
                            rec = work.tile([128, QCH], f32, tag="rec")
                            nc.vector.reciprocal_approx_fast(rec, lrep)
                            nc.vector.tensor_mul(
                                uT[:, h, q0:q0 + QCH], outp, rec
                            )
                            yield
                        p3q.append((bi, qc))

                # seg1: projections for batch 0 alone
                _run_interleaved((p1_gen(0, first=True), 1))
                # seg2: projections(b1) interleaved with attention(b0)
                _run_interleaved((p1_gen(1), 1), (attn_gen(0), 1))

            # p1 PSUM pools closed; banks free for phase 3
            with tc.tile_pool(name="ps3", bufs=2, space="PSUM") as ps3:
                # ------------- phase 3: output projection -------------
                # consumes ready 512-column slabs from p3q; a slab (bi, qc)
                # is pushed once both heads of that q-chunk wrote uT

                def p3_slab(bi, qc, ceng):
                    c0 = bi * s + qc * QCH
                    for oc in range(dim // 128):
                        pos = ps3.tile([128, QCH], f32, tag="p3", bufs=4)
                        for h in range(HPC):
                            nc.tensor.matmul(
                                pos,
                                lhsT=woT_s[:, h, oc * 128:(oc + 1) * 128],
                                rhs=uT[:, h, c0:c0 + QCH],
                                start=(h == 0),
                                stop=(h == HPC - 1),
                            )
                        ot = otp.tile([128, QCH], bf16, tag="ot", bufs=6)
                        eng = ceng[oc % len(ceng)]
                        if eng is nc.scalar:
                            eng.copy(ot, pos)
                        else:
                            eng.tensor_copy(ot, pos)
                        yield
                        # split write issue across the sync and gpsimd queues
                        deng = nc.sync if oc % 2 == 0 else nc.gpsimd
                        deng.dma_start(out=out_d[oc, :, c0:c0 + QCH], in_=ot)
                        yield

                def p3_consumer(n_slabs, ceng):
                    served = 0
                    while served < n_slabs:
                        if not p3q:
                            yield  # waiting on attention progress
                            continue
                        bi, qc = p3q.pop(0)
                        yield from p3_slab(bi, qc, ceng)
                        served += 1

                # seg3: attention(b1) interleaved with all ready output-
                # projection slabs (b0 now, b1 as q-chunks complete);
                # GpSimd cannot access PSUM, and Act owns the exps, so
                # copies lean on DVE
                _run_interleaved(
                    (attn_gen(1), 1),
                    (p3_consumer(2 * NQC - 1, [nc.vector, nc.vector, nc.scalar]), 1),
                )
                # tail: the final slab has nothing left to overlap with;
                # alternate copies over the now-idle Act and DVE
                _run_interleaved((p3_consumer(1, [nc.scalar, nc.vector]), 1))

    nc.compile()
    return nc


def make_in_maps(x, Wq, Wk, Wv, Wo, b=B, s=S, dim=DIM, n_cores=N_CORES):
    import ml_dtypes

    bf = ml_dtypes.bfloat16
    f8 = ml_dtypes.float8_e4m3fn
    bs = b * s
    xT = np.ascontiguousarray(x.reshape(bs, dim).T)          # [dim, bs] f32
    # [g, p, c*SC1+t]: per-512-token chunk, contiguous per partition
    xP4 = np.ascontiguousarray(
        xT.reshape(NDIN, 128, bs // SC1, SC1).transpose(2, 1, 0, 3)
    ).reshape(bs // SC1, 128, NDIN * SC1)
    xP = xP4.astype(bf)
    x8P = xP4.astype(f8)
    cosT1, sinT1 = _rope_tables_T(s, HD)
    cosT = np.ascontiguousarray(np.tile(cosT1, (1, b))).astype(bf)
    sinT = np.ascontiguousarray(np.tile(sinT1, (1, b))).astype(bf)
    rT = _rot_matrix_T(HD).astype(bf)
    ones = np.ones((HD, HD), dtype=np.float32).astype(bf)
    tri = _tri_mask().astype(bf)

    def wprep(wT):  # [dim, DLOC] -> [128, NDIN*DLOC] contiguous rows
        return np.ascontiguousarray(
            wT.reshape(NDIN, 128, DLOC).transpose(1, 0, 2)
        ).reshape(128, NDIN * DLOC)

    in_maps = []
    for c in range(n_cores):
        sl = slice(c * DLOC, (c + 1) * DLOC)
        woT = np.ascontiguousarray(Wo[:, sl].T)  # [DLOC, dim]
        woP = np.ascontiguousarray(
            woT.reshape(HPC, 128, dim).transpose(1, 0, 2)
        ).reshape(128, HPC * dim)
        in_maps.append(
            {
                "xP": xP,
                "x8P": x8P,
                "wq8": wprep(
                    np.ascontiguousarray(Wq[sl, :].T) * WSCALE
                ).astype(f8),
                "wk8": wprep(
                    np.ascontiguousarray(Wk[sl, :].T) * WSCALE
                ).astype(f8),
                "wqb": wprep(
                    np.ascontiguousarray(Wq[sl, :].T) * WSCALE
                ).astype(bf),
                "wkb": wprep(
                    np.ascontiguousarray(Wk[sl, :].T) * WSCALE
                ).astype(bf),
                "wvP": wprep(np.ascontiguousarray(Wv[sl, :].T)).astype(bf),
                "woP": woP.astype(bf),
                "cosT": cosT,
                "sinT": sinT,
                "rT": rT,
                "ones": ones,
                "tri": tri,
            }
        )
    return in_maps


def kernel(x, Wq, Wk, Wv, Wo, _trace=False):
    """Full-input / full-output entry point. Shards over 8 cores internally."""
    if "/opt/trn_rl_repo" not in sys.path:
        sys.path.insert(0, "/opt/trn_rl_repo")
    from concourse.bass_utils import run_bass_kernel_spmd

    x = np.asarray(x, dtype=np.float32)
    Wq, Wk, Wv, Wo = (np.asarray(w, dtype=np.float32) for w in (Wq, Wk, Wv, Wo))

    key = (B, S, DIM)
    if key not in _PROGRAM_CACHE:
        _PROGRAM_CACHE[key] = build_program(B, S, DIM)
    nc = _PROGRAM_CACHE[key]

    in_maps = make_in_maps(x, Wq, Wk, Wv, Wo)
    res = run_bass_kernel_spmd(
        nc, in_maps, core_ids=list(range(N_CORES)), trace=_trace
    )
    kernel.last_results = res
    acc = res.results[0]["out"].astype(np.float32)
    for c in range(1, N_CORES):
        acc = acc + res.results[c]["out"].astype(np.float32)
    # out is [16, 128, bs] oc-major; flatten to [dim, bs] then to [B, S, DIM]
    return np.ascontiguousarray(acc.reshape(DIM, B * S).T).reshape(B, S, DIM)


# revision 32
# speedup vs baseline: 1.5491x; 1.0057x over previous
"""Multi-head causal self-attention with RoPE, tensor-parallel over heads
across 8 Trainium2 NeuronCores.

Strategy (Megatron-style TP over heads), v4:
  - Each core owns 2 of the 16 heads: rows [c*256,(c+1)*256) of Wq/Wk/Wv
    and the matching columns of Wo. Host sums the 8 partial outputs.
  - q/k projections run in fp8e4m3 with DoubleRow perf mode (0.5
    cycles/row, 2x PE throughput); the host pre-scales Wq/Wk by 64 to
    center the fp8 dynamic range, and the 64*64 factor is folded into
    the exp scale. The q/k quantization noise washes out through the
    softmax (diffuse attention); the value path (v, Wo) stays bf16.
  - Everything else in bf16 (PSUM accumulation fp32).
  - All DRAM operands are host-prearranged so every DMA is contiguous
    per partition (128 descriptors, cheap HWDGE issue); out is written
    oc-major [16, 128, bs] and reassembled on host.
  - Batch-pipelined generator emission: p1(b0) -> [p1(b1) || attn(b0)]
    -> [attn(b1) || p3(ready slabs)] -> p3(last slab). Independent
    matmuls fill the in-order PE queue between each score-matmul ->
    exp -> AV-matmul dependency chain.
  - Causal: off-diagonal k-chunks full-width; the 4 diagonal chunks of
    each 512-wide q-chunk use a reduced column range plus a 128x128
    additive triangle mask.
"""

import sys

import numpy as np

B, S, DIM = 2, 2048, 2048
NUM_HEADS = 16
HD = 128
N_CORES = 8
HPC = NUM_HEADS // N_CORES  # heads per core
DLOC = HPC * HD             # per-core slice of the model dim
ROPE_BASE = 10000.0
NDIN = DIM // 128           # contraction chunks for projections
SC1 = 512                   # phase-1 s-chunk
QCH = 512                   # attention q-chunk
NQC = S // QCH              # q-chunks per batch
NS1B = S // SC1             # phase-1 s-chunks per batch
WSCALE = 64.0               # fp8 pre-scale on Wq/Wk

_PROGRAM_CACHE = {}


def _rope_tables_T(seq_len, head_dim):
    # match reference float32 arithmetic: inv_freq over even indices,
    # emb = cat(freqs, freqs); returned transposed [head_dim, seq_len]
    inv_freq = (
        1.0
        / (np.float32(ROPE_BASE)
           ** (np.arange(0, head_dim, 2, dtype=np.float32) / np.float32(head_dim)))
    ).astype(np.float32)
    t = np.arange(seq_len, dtype=np.float32)
    freqs = np.outer(t, inv_freq).astype(np.float32)      # [S, D/2]
    emb = np.concatenate([freqs, freqs], axis=-1)         # [S, D]
    return (
        np.ascontiguousarray(np.cos(emb).astype(np.float32).T),
        np.ascontiguousarray(np.sin(emb).astype(np.float32).T),
    )


def _rot_matrix_T(head_dim):
    # rotated = cat(-x[1::2], x[::2]) = R @ x; return R.T [D, D]
    d2 = head_dim // 2
    R = np.zeros((head_dim, head_dim), dtype=np.float32)
    for dp in range(d2):
        R[dp, 2 * dp + 1] = -1.0
    for dp in range(d2, head_dim):
        R[dp, 2 * (dp - d2)] = 1.0
    return np.ascontiguousarray(R.T)


def _tri_mask():
    # tri[kk, qq] = 0 if kk <= qq else -1e9 (additive, pre-exp-scale)
    kk = np.arange(128)[:, None]
    qq = np.arange(128)[None, :]
    return np.where(kk <= qq, 0.0, -1e9).astype(np.float32)


def _run_interleaved(*specs):
    """specs: (generator, steps_per_round). Round-robin until exhausted."""
    live = [[iter(g), k] for g, k in specs]
    while live:
        for item in list(live):
            g, k = item
            try:
                for _ in range(k):
                    next(g)
            except StopIteration:
                live.remove(item)


def build_program(b=B, s=S, dim=DIM):
    """Builds the per-core SPMD Bass program (identical on every core)."""
    if "/opt/trn_rl_repo" not in sys.path:
        sys.path.insert(0, "/opt/trn_rl_repo")
    import concourse.bacc as bacc
    import concourse.mybir as mybir
    import concourse.tile as tile

    f32 = mybir.dt.float32
    bf16 = mybir.dt.bfloat16
    fp8 = mybir.dt.float8e4
    EXP = mybir.ActivationFunctionType.Exp
    DR = mybir.MatmulPerfMode.DoubleRow

    bs = b * s
    scale = float(HD) ** -0.5 / (WSCALE * WSCALE)
    nsc = s // 128   # 128-token chunks per batch
    ngc = bs // SC1  # global 512-token chunks

    nc = bacc.Bacc("TRN2", target_bir_lowering=False, debug=False)

    # host-prearranged layouts: contiguous per partition
    xP_d = nc.dram_tensor("xP", [ngc, 128, NDIN * SC1], bf16, kind="ExternalInput")
    x8P_d = nc.dram_tensor("x8P", [ngc, 128, NDIN * SC1], fp8, kind="ExternalInput")
    wq8_d = nc.dram_tensor("wq8", [128, NDIN * DLOC], fp8, kind="ExternalInput")
    wk8_d = nc.dram_tensor("wk8", [128, NDIN * DLOC], fp8, kind="ExternalInput")
    wqb_d = nc.dram_tensor("wqb", [128, NDIN * DLOC], bf16, kind="ExternalInput")
    wkb_d = nc.dram_tensor("wkb", [128, NDIN * DLOC], bf16, kind="ExternalInput")
    wvP_d = nc.dram_tensor("wvP", [128, NDIN * DLOC], bf16, kind="ExternalInput")
    woP_d = nc.dram_tensor("woP", [128, HPC * dim], bf16, kind="ExternalInput")
    cosT_d = nc.dram_tensor("cosT", [HD, bs], bf16, kind="ExternalInput")
    sinT_d = nc.dram_tensor("sinT", [HD, bs], bf16, kind="ExternalInput")
    rT_d = nc.dram_tensor("rT", [HD, HD], bf16, kind="ExternalInput")
    ones_d = nc.dram_tensor("ones", [HD, HD], bf16, kind="ExternalInput")
    tri_d = nc.dram_tensor("tri", [HD, HD], bf16, kind="ExternalInput")
    out_d = nc.dram_tensor("out", [dim // 128, 128, bs], bf16, kind="ExternalOutput")

    with tile.TileContext(nc) as tc:
        with (
            tc.tile_pool(name="persist", bufs=1) as persist,
            tc.tile_pool(name="work", bufs=2) as work,
            tc.tile_pool(name="ptp", bufs=4) as ptp,
            tc.tile_pool(name="otp", bufs=3) as otp,
            tc.tile_pool(name="ps_at", bufs=1, space="PSUM") as ps_at,
        ):
            # ---- persistent tiles ----
            qT = persist.tile([128, HPC, bs], bf16)   # roped q (x64), [d, h, tok]
            kT = persist.tile([128, HPC, bs], bf16)
            vS = persist.tile([128, bs // 128, DLOC], bf16)  # [tok, chunk, d]
            uT = persist.tile([128, HPC, bs], bf16)   # attention out
            wq_s = persist.tile([128, NDIN, DLOC], fp8)
            wk_s = persist.tile([128, NDIN, DLOC], fp8)
            # bf16 copies for the first 512 tokens of each batch: early
            # causal rows have concentrated attention, so fp8 q/k noise
            # doesn't wash out there (x64-scaled like the fp8 path)
            wqb_s = persist.tile([128, NDIN, DLOC], bf16)
            wkb_s = persist.tile([128, NDIN, DLOC], bf16)
            wv_s = persist.tile([128, NDIN, DLOC], bf16)
            woT_s = persist.tile([128, HPC, dim], bf16)
            cosS = persist.tile([128, bs], bf16)
            sinS = persist.tile([128, bs], bf16)
            rTs = persist.tile([HD, HD], bf16)
            onesS = persist.tile([HD, HD], bf16)
            triS = persist.tile([HD, HD], bf16)

            # ---- startup DMAs: critical first, split across queues ----
            # first chain (si=0, bf16 path) needs wqb + xt interleaved in
            # consumption order on sync; everything else on scalar/gpsimd
            def wview(wd):
                return wd.rearrange("p (c m) -> p c m", c=NDIN)

            nc.sync.dma_start(out=wqb_s[:, 0:8, :], in_=wview(wqb_d)[:, 0:8, :])
            nc.scalar.dma_start(out=wkb_s, in_=wview(wkb_d))
            nc.gpsimd.dma_start(out=rTs, in_=rT_d[:])
            nc.gpsimd.dma_start(out=onesS, in_=ones_d[:])
            nc.gpsimd.dma_start(out=triS, in_=tri_d[:])

            def startup_x0(xt, xsrc):
                # split the first x chunk across the sync and gpsimd queues
                # so the first bf16 qk chain unblocks ~2x sooner
                nc.sync.dma_start(out=xt[:, 0:4, :], in_=xsrc[:, 0:4, :])
                nc.gpsimd.dma_start(out=xt[:, 8:12, :], in_=xsrc[:, 8:12, :])
                nc.sync.dma_start(out=wqb_s[:, 8:16, :], in_=wview(wqb_d)[:, 8:16, :])
                nc.sync.dma_start(out=xt[:, 4:8, :], in_=xsrc[:, 4:8, :])
                nc.gpsimd.dma_start(out=xt[:, 12:16, :], in_=xsrc[:, 12:16, :])

            def startup_rest():
                # nothing PE-side in seg1 depends on cos/sin (RoPE runs on
                # DVE and only feeds the qT/kT stores read in seg2), so the
                # fp8 weights can jump the queue
                nc.scalar.dma_start(out=wv_s, in_=wview(wvP_d))
                nc.scalar.dma_start(out=wq_s, in_=wview(wq8_d))
                nc.scalar.dma_start(out=wk_s, in_=wview(wk8_d))
                nc.scalar.dma_start(out=cosS, in_=cosT_d[:])
                nc.scalar.dma_start(out=sinS, in_=sinT_d[:])
                # phase-3 weights: needed only after attn(b0); issue last
                nc.scalar.dma_start(
                    out=woT_s, in_=woP_d.rearrange("p (h n) -> p h n", h=HPC)
                )

            with (
                tc.tile_pool(name="p1x", bufs=2) as p1x,
                tc.tile_pool(name="p1ps", bufs=1, space="PSUM") as p1ps,
            ):
                # ------------- phase 1: qkv projections + RoPE -------------
                def p1_gen(bi, first=False):
                    for si in range(NS1B):
                        s0 = bi * s + si * SC1
                        g = s0 // SC1
                        xt = p1x.tile([128, NDIN, SC1], bf16, tag="xt")
                        xsrc = xP_d[g].rearrange("p (c t) -> p c t", c=NDIN)
                        if si == 0:
                            # bf16 q/k path for the first 512 tokens
                            xt8 = None
                            if first:
                                startup_x0(xt, xsrc)
                                startup_rest()
                            else:
                                nc.sync.dma_start(out=xt[:, :8, :], in_=xsrc[:, :8, :])
                                nc.sync.dma_start(out=xt[:, 8:, :], in_=xsrc[:, 8:, :])
                        else:
                            xt8 = p1x.tile([128, NDIN, SC1], fp8, tag="xt8")
                            nc.sync.dma_start(
                                out=xt8,
                                in_=x8P_d[g].rearrange("p (c t) -> p c t", c=NDIN),
                            )
                            nc.sync.dma_start(out=xt[:, :8, :], in_=xsrc[:, :8, :])
                            nc.sync.dma_start(out=xt[:, 8:, :], in_=xsrc[:, 8:, :])
                        yield
                        vq = [0, 1, 2, 3]

                        def v_chain(sub):
                            vacc = p1ps.tile([128, DLOC], f32, tag="v", bufs=1)
                            for c in range(NDIN):
                                nc.tensor.matmul(
                                    vacc,
                                    lhsT=xt[:, c, sub * 128:(sub + 1) * 128],
                                    rhs=wv_s[:, c, :],
                                    start=(c == 0),
                                    stop=(c == NDIN - 1),
                                )
                                if c % 4 == 3:
                                    yield
                            nc.scalar.copy(vS[:, s0 // 128 + sub, :], vacc)
                            yield

                        ci = 0
                        wpairs = (
                            ((wqb_s, qT), (wkb_s, kT)) if si == 0
                            else ((wq_s, qT), (wk_s, kT))
                        )
                        for w_t, store in wpairs:
                            for h in range(HPC):
                                acc = p1ps.tile([128, SC1], f32, tag="qk", bufs=2)
                                if si == 0:
                                    for c in range(NDIN):
                                        nc.tensor.matmul(
                                            acc,
                                            lhsT=w_t[:, c, h * HD:(h + 1) * HD],
                                            rhs=xt[:, c, :],
                                            start=(c == 0),
                                            stop=(c == NDIN - 1),
                                        )
                                        if c % 4 == 3:
                                            yield
                                else:
                                    for c2 in range(0, NDIN, 2):
                                        nc.tensor.matmul(
                                            acc,
                                            lhsT=w_t[:, c2:c2 + 2, h * HD:(h + 1) * HD],
                                            rhs=xt8[:, c2:c2 + 2, :],
                                            start=(c2 == 0),
                                            stop=(c2 == NDIN - 2),
                                            perf_mode=DR,
                                        )
                                        if c2 % 4 == 2:
                                            yield
                                raw = p1x.tile([128, SC1], bf16, tag="raw")
                                nc.vector.tensor_copy(raw, acc)
                                rot = p1ps.tile([128, SC1], f32, tag="rot", bufs=1)
                                nc.tensor.matmul(
                                    rot, lhsT=rTs, rhs=raw, start=True, stop=True
                                )
                                yield
                                t1 = p1x.tile([128, SC1], bf16, tag="t1")
                                nc.vector.tensor_mul(t1, raw, cosS[:, s0:s0 + SC1])
                                t2 = p1x.tile([128, SC1], bf16, tag="t2")
                                nc.vector.tensor_mul(t2, rot, sinS[:, s0:s0 + SC1])
                                nc.vector.tensor_add(
                                    store[:, h, s0:s0 + SC1], t1, t2
                                )
                                yield
                                ci += 1
                                # delay v chains past the first two qk chains
                                # (wv lands late in the startup order)
                                if vq and ci >= 2:
                                    yield from v_chain(vq.pop(0))
                        while vq:
                            yield from v_chain(vq.pop(0))

                # ------------- phase 2: causal attention -------------
                # p3q: output-projection slabs whose uT columns are complete
                p3q = []

                def attn_gen(bi):
                    # NOTE: pt must stay bf16 — real scores reach 9.08, so
                    # exp hits 8.8e3, far over fp8-e4m3's 448 max (NaN)
                    for qc in range(NQC):
                        for h in range(HPC):
                            q0 = bi * s + qc * QCH
                            nkc = (qc + 1) * QCH // 128
                            outp = ps_at.tile([128, QCH], f32, tag="o", bufs=1)
                            lrep = ps_at.tile([128, QCH], f32, tag="l", bufs=1)
                            npair = (nkc - 4) // 2
                            nones = npair + 4
                            li = 0
                            prev_pt = None
                            for kc in range(nkc):
                                k0 = bi * s + kc * 128
                                di = kc - (nkc - 4)
                                lo = max(0, di * 128)
                                st = ps_at.tile([128, QCH], f32, tag="st", bufs=2)
                                nc.tensor.matmul(
                                    st[:, lo:],
                                    lhsT=kT[:, h, k0:k0 + 128],
                                    rhs=qT[:, h, q0 + lo:q0 + QCH],
                                    start=True,
                                    stop=True,
                                )
                                if di >= 0:
                                    nc.vector.tensor_add(
                                        st[:, lo:lo + 128], st[:, lo:lo + 128], triS
                                    )
                                pt = ptp.tile([128, QCH], bf16, tag="pt")
                                nc.scalar.activation(
                                    pt[:, lo:], st[:, lo:], EXP, scale=scale
                                )
                                yield
                                nc.tensor.matmul(
                                    outp[:, lo:],
                                    lhsT=vS[:, bi * nsc + kc, h * HD:(h + 1) * HD],
                                    rhs=pt[:, lo:],
                                    start=(kc == 0),
                                    stop=(kc == nkc - 1),
                                )
                                if di >= 0:
                                    # diagonal chunks: single range-reduced
                                    # denominator matmul
                                    nc.tensor.matmul(
                                        lrep[:, lo:],
                                        lhsT=onesS,
                                        rhs=pt[:, lo:],
                                        start=(li == 0),
                                        stop=(li == nones - 1),
                                    )
                                    li += 1
                                elif kc % 2 == 1:
                                    # off-diagonal: pair-sum on DVE/GpSimd,
                                    # then one denominator matmul per pair
                                    lp = work.tile([128, QCH], bf16, tag="lp")
                                    eng = nc.vector if li % 2 == 0 else nc.gpsimd
                                    eng.tensor_add(lp, prev_pt, pt)
                                    nc.tensor.matmul(
                                        lrep,
                                        lhsT=onesS,
                                        rhs=lp,
                                        start=(li == 0),
                                        stop=(li == nones - 1),
                                    )
                                    li += 1
                                prev_pt = pt
                                yield
                            rec = work.tile([128, QCH], f32, tag="rec")
                            nc.vector.reciprocal_approx_fast(rec, lrep)
                            nc.vector.tensor_mul(
                                uT[:, h, q0:q0 + QCH], outp, rec
                            )
                            yield
                        p3q.append((bi, qc))

                # seg1: projections for batch 0 alone
                _run_interleaved((p1_gen(0, first=True), 1))
                # seg2: projections(b1) interleaved with attention(b0)
                _run_interleaved((p1_gen(1), 1), (attn_gen(0), 1))

            # p1 PSUM pools closed; banks free for phase 3
            with tc.tile_pool(name="ps3", bufs=2, space="PSUM") as ps3:
                # ------------- phase 3: output projection -------------
                # consumes ready 512-column slabs from p3q; a slab (bi, qc)
                # is pushed once both heads of that q-chunk wrote uT

                def p3_slab(bi, qc, ceng):
                    c0 = bi * s + qc * QCH
                    for oc in range(dim // 128):
                        pos = ps3.tile([128, QCH], f32, tag="p3", bufs=4)
                        for h in range(HPC):
                            nc.tensor.matmul(
                                pos,
                                lhsT=woT_s[:, h, oc * 128:(oc + 1) * 128],
                                rhs=uT[:, h, c0:c0 + QCH],
                                start=(h == 0),
                                stop=(h == HPC - 1),
                            )
                        ot = otp.tile([128, QCH], bf16, tag="ot", bufs=6)
                        eng = ceng[oc % len(ceng)]
                        if eng is nc.scalar:
                            eng.copy(ot, pos)
                        else:
                            eng.tensor_copy(ot, pos)
                        yield
                        # split write issue across the sync and gpsimd queues
                        deng = nc.sync if oc % 2 == 0 else nc.gpsimd
                        deng.dma_start(out=out_d[oc, :, c0:c0 + QCH], in_=ot)
                        yield

                def p3_consumer(n_slabs, ceng):
                    served = 0
                    while served < n_slabs:
                        if not p3q:
                            yield  # waiting on attention progress
                            continue
                        bi, qc = p3q.pop(0)
                        yield from p3_slab(bi, qc, ceng)
                        served += 1

                # seg3: attention(b1) interleaved with all ready output-
                # projection slabs (b0 now, b1 as q-chunks complete);
                # GpSimd cannot access PSUM, and Act owns the exps, so
                # copies lean on DVE
                _run_interleaved(
                    (attn_gen(1), 1),
                    (p3_consumer(2 * NQC - 1, [nc.vector, nc.vector, nc.scalar]), 1),
                )
                # tail: the final slab has nothing left to overlap with;
                # alternate copies over the now-idle Act and DVE
                _run_interleaved((p3_consumer(1, [nc.scalar, nc.vector]), 1))

    nc.compile()
    return nc


def make_in_maps(x, Wq, Wk, Wv, Wo, b=B, s=S, dim=DIM, n_cores=N_CORES):
    import ml_dtypes

    bf = ml_dtypes.bfloat16
    f8 = ml_dtypes.float8_e4m3fn
    bs = b * s
    xT = np.ascontiguousarray(x.reshape(bs, dim).T)          # [dim, bs] f32
    # [g, p, c*SC1+t]: per-512-token chunk, contiguous per partition
    xP4 = np.ascontiguousarray(
        xT.reshape(NDIN, 128, bs // SC1, SC1).transpose(2, 1, 0, 3)
    ).reshape(bs // SC1, 128, NDIN * SC1)
    xP = xP4.astype(bf)
    x8P = xP4.astype(f8)
    cosT1, sinT1 = _rope_tables_T(s, HD)
    cosT = np.ascontiguousarray(np.tile(cosT1, (1, b))).astype(bf)
    sinT = np.ascontiguousarray(np.tile(sinT1, (1, b))).astype(bf)
    rT = _rot_matrix_T(HD).astype(bf)
    ones = np.ones((HD, HD), dtype=np.float32).astype(bf)
    tri = _tri_mask().astype(bf)

    def wprep(wT):  # [dim, DLOC] -> [128, NDIN*DLOC] contiguous rows
        return np.ascontiguousarray(
            wT.reshape(NDIN, 128, DLOC).transpose(1, 0, 2)
        ).reshape(128, NDIN * DLOC)

    in_maps = []
    for c in range(n_cores):
        sl = slice(c * DLOC, (c + 1) * DLOC)
        woT = np.ascontiguousarray(Wo[:, sl].T)  # [DLOC, dim]
        woP = np.ascontiguousarray(
            woT.reshape(HPC, 128, dim).transpose(1, 0, 2)
        ).reshape(128, HPC * dim)
        in_maps.append(
            {
                "xP": xP,
                "x8P": x8P,
                "wq8": wprep(
                    np.ascontiguousarray(Wq[sl, :].T) * WSCALE
                ).astype(f8),
                "wk8": wprep(
                    np.ascontiguousarray(Wk[sl, :].T) * WSCALE
                ).astype(f8),
                "wqb": wprep(
                    np.ascontiguousarray(Wq[sl, :].T) * WSCALE
                ).astype(bf),
                "wkb": wprep(
                    np.ascontiguousarray(Wk[sl, :].T) * WSCALE
                ).astype(bf),
                "wvP": wprep(np.ascontiguousarray(Wv[sl, :].T)).astype(bf),
                "woP": woP.astype(bf),
                "cosT": cosT,
                "sinT": sinT,
                "rT": rT,
                "ones": ones,
                "tri": tri,
            }
        )
    return in_maps


def kernel(x, Wq, Wk, Wv, Wo, _trace=False):
    """Full-input / full-output entry point. Shards over 8 cores internally."""
    if "/opt/trn_rl_repo" not in sys.path:
        sys.path.insert(0, "/opt/trn_rl_repo")
    from concourse.bass_utils import run_bass_kernel_spmd

    x = np.asarray(x, dtype=np.float32)
    Wq, Wk, Wv, Wo = (np.asarray(w, dtype=np.float32) for w in (Wq, Wk, Wv, Wo))

    key = (B, S, DIM)
    if key not in _PROGRAM_CACHE:
        _PROGRAM_CACHE[key] = build_program(B, S, DIM)
    nc = _PROGRAM_CACHE[key]

    in_maps = make_in_maps(x, Wq, Wk, Wv, Wo)
    res = run_bass_kernel_spmd(
        nc, in_maps, core_ids=list(range(N_CORES)), trace=_trace
    )
    kernel.last_results = res
    acc = res.results[0]["out"].astype(np.float32)
    for c in range(1, N_CORES):
        acc = acc + res.results[c]["out"].astype(np.float32)
    # out is [16, 128, bs] oc-major; flatten to [dim, bs] then to [B, S, DIM]
    return np.ascontiguousarray(acc.reshape(DIM, B * S).T).reshape(B, S, DIM)
